# revision 12
# baseline (speedup 1.0000x reference)
"""Trainium2 Bass kernel for nn_MultiHeadAttention_28028956574019.

Sparse windowed multi-head attention, G=4 window groups, learned per-row
window offset. Data-parallel over batch: 8 NeuronCores, one batch element
per core.

Per-core device program (L=2048, H=1024, d=256 per group):
  phase 1: learned-offset path dx = L*sigmoid(lin2(relu(q) @ off_w.T))
           computed in fp32r; mask row W = BIG*(q_idx + dx) broadcast to
           [128, 2048] via gpsimd partition_broadcast.
  phase 2: Q/K projection (bf16): Q_T/K_T stored [h, l] (transposed).
  phase 3: V projection (bf16): V stored [l, h] (natural).
  phase 4: per group, per 512-wide q-strip, k-blocks descending 15..4s:
           S_T[k,q] = K_T.T@Q_T (PSUM f32), masked via
           Z = min(S, W - BIG*(k - ws)) [one DVE scalar_tensor_tensor]
           + diagonal lower-bound tile min, p = exp(SCALE2*Z) -> bf16,
           denominator via ones-matmul (PSUM accumulate), out += p.T@V.
           Normalization deferred: denom row DMA round-trips through DRAM
           to per-partition layout, out_raw * recip(denom).

All shapes hardcoded for the fixed problem size. The harness calls
kernel(**inputs) with the full (unsharded) inputs.
"""

import sys

if "/opt/trn_rl_repo" not in sys.path:
    sys.path.insert(0, "/opt/trn_rl_repo")

import numpy as np
import ml_dtypes

import concourse.bass as bass  # noqa: F401  (bass must import before bacc)
from concourse import bacc
import concourse.mybir as mybir
from concourse.tile import TileContext
from concourse.bass_utils import run_bass_kernel_spmd

dt = mybir.dt
AF = mybir.ActivationFunctionType
Alu = mybir.AluOpType

B, L, H = 8, 2048, 1024
G, D = 4, 256          # groups, per-group head dim
D1 = 256               # learned-offset hidden dim
WS = [4, 16, 64, 256]
BIG = 1.0e7
SCALE2 = 2.0 / float(np.sqrt(L))   # masked_fill+add doubles unmasked scores
NCORES = 8

_BUILT = None


def build_nc(trace_friendly=False):
    nc = bacc.Bacc("TRN2", target_bir_lowering=False, debug=False)

    # ---- I/O ----
    qt32 = nc.declare_dram_parameter("qt32", [H, L], dt.float32, isOutput=False)
    qtbf = nc.declare_dram_parameter("qtbf", [H, L], dt.bfloat16, isOutput=False)
    wqk = nc.declare_dram_parameter("wqk", [H, 2 * H], dt.bfloat16, isOutput=False)
    wv = nc.declare_dram_parameter("wv", [H, H], dt.bfloat16, isOutput=False)
    woff = nc.declare_dram_parameter("woff", [H, D1], dt.float32, isOutput=False)
    wlin2 = nc.declare_dram_parameter("wlin2", [D1, 1], dt.float32, isOutput=False)
    bqk = nc.declare_dram_parameter("bqk", [128, 16], dt.float32, isOutput=False)
    bv = nc.declare_dram_parameter("bv", [128, H], dt.bfloat16, isOutput=False)
    blin2 = nc.declare_dram_parameter("blin2", [1, 1], dt.float32, isOutput=False)
    dtile = nc.declare_dram_parameter("dtile", [128, 128], dt.float32, isOutput=False)
    iotab = nc.declare_dram_parameter("iotab", [1, L], dt.float32, isOutput=False)
    kvec = nc.declare_dram_parameter("kvec", [128, 64], dt.float32, isOutput=False)
    out = nc.declare_dram_parameter("out", [L, H], dt.float32, isOutput=True)

    dend = nc.dram_tensor("dend", [16, 512], dt.float32)

    with TileContext(nc) as tc:
        with tc.tile_pool(name="persist", bufs=1) as pp:
            # bf16 query (transposed), resident for phases 2-3
            qtb = []
            for i in range(8):
                t = pp.tile([128, L], dt.bfloat16, name=f"qtb{i}")
                nc.gpsimd.dma_start(out=t[:], in_=qtbf[i * 128:(i + 1) * 128, :])
                qtb.append(t)
            # bf16 V-projection weights, resident
            wv_t = []
            for i in range(8):
                t = pp.tile([128, H], dt.bfloat16, name=f"wv{i}")
                nc.gpsimd.dma_start(out=t[:], in_=wv[i * 128:(i + 1) * 128, :])
                wv_t.append(t)

            # ---- consts ----
            dt_t = pp.tile([128, 128], dt.float32, name="dt_t")
            nc.gpsimd.dma_start(out=dt_t[:], in_=dtile[:])
            kvec_t = pp.tile([128, 64], dt.float32, name="kvec_t")
            nc.gpsimd.dma_start(out=kvec_t[:], in_=kvec[:])
            bqk_t = pp.tile([128, 16], dt.float32, name="bqk_t")
            nc.gpsimd.dma_start(out=bqk_t[:], in_=bqk[:])
            blin2_t = pp.tile([1, 1], dt.float32, name="blin2_t")
            nc.gpsimd.dma_start(out=blin2_t[:], in_=blin2[:])
            bvb = pp.tile([128, H], dt.bfloat16, name="bvb")
            nc.gpsimd.dma_start(out=bvb[:], in_=bv[:])
            ones_t = pp.tile([128, 1], dt.bfloat16, name="ones_t")
            nc.vector.memset(ones_t[:], 1.0)
            wbig = pp.tile([128, L], dt.float32, name="wbig")

            # persistent Q_T / K_T / V
            QT = [[pp.tile([128, L], dt.bfloat16, name=f"QT{g}{h}", tag=f"QT{g}{h}") for h in range(2)]
                  for g in range(G)]
            KT = [[pp.tile([128, L], dt.bfloat16, name=f"KT{g}{h}", tag=f"KT{g}{h}") for h in range(2)]
                  for g in range(G)]
            VT = [pp.tile([128, H], dt.bfloat16, name=f"VT{lb}", tag=f"VT{lb}") for lb in range(16)]

            # ================= phase 2: Q/K projection =================
            with tc.tile_pool(name="p2", bufs=1) as p2, \
                 tc.tile_pool(name="ps2", bufs=3, space="PSUM") as ps2:
                for hb in range(16):
                    wt = []
                    for hin in range(8):
                        t = p2.tile([128, 128], dt.bfloat16, tag="wqk", bufs=18)
                        nc.sync.dma_start(
                            out=t[:],
                            in_=wqk[hin * 128:(hin + 1) * 128, hb * 128:(hb + 1) * 128])
                        wt.append(t)
                    g, h = (hb % 8) // 2, hb % 2
                    dest = QT[g][h] if hb < 8 else KT[g][h]
                    for s in range(4):
                        pps = ps2.tile([128, 512], dt.float32, tag="qkps")
                        for hin in range(8):
                            nc.tensor.matmul(pps[:], wt[hin][:],
                                             qtb[hin][:, s * 512:(s + 1) * 512],
                                             start=(hin == 0), stop=(hin == 7))
                        nc.scalar.activation(dest[:, s * 512:(s + 1) * 512], pps[:],
                                             AF.Identity, bias=bqk_t[:, hb:hb + 1],
                                             scale=1.0)

            # ================= phase 1: dx path =================
            with tc.tile_pool(name="p1", bufs=1) as p1, \
                 tc.tile_pool(name="ps1", bufs=2, space="PSUM") as ps1:
                woff_t = []
                for i in range(8):
                    t = p1.tile([128, D1], dt.float32r, name=f"woff{i}")
                    nc.gpsimd.dma_start(out=t[:],
                                      in_=woff[i * 128:(i + 1) * 128, :].bitcast(dt.float32r))
                    woff_t.append(t)
                wlin_t = []
                for i in range(2):
                    t = p1.tile([128, 1], dt.float32r, name=f"wlin{i}")
                    nc.gpsimd.dma_start(out=t[:],
                                      in_=wlin2[i * 128:(i + 1) * 128, :].bitcast(dt.float32r))
                    wlin_t.append(t)
                sig_row = p1.tile([1, L], dt.float32, name="sig_row")
                iotab_t = p1.tile([1, L], dt.float32, name="iotab_t")
                nc.gpsimd.dma_start(out=iotab_t[:], in_=iotab[:])

                qs_tiles = {}
                for s in range(4):
                    for hin in range(8):
                        q_s = p1.tile([128, 512], dt.float32, tag="qs", bufs=8,
                                      name="q_s")
                        nc.gpsimd.dma_start(
                            out=q_s[:],
                            in_=qt32[hin * 128:(hin + 1) * 128, s * 512:(s + 1) * 512])
                        qs_tiles[s, hin] = q_s
                for s in range(4):
                    t1ps_a = ps1.tile([128, 512], dt.float32, tag="t1psa")
                    t1ps_b = ps1.tile([128, 512], dt.float32, tag="t1psb")
                    for hin in range(8):
                        q_s = qs_tiles[s, hin]
                        relu_s = p1.tile([128, 512], dt.float32r, tag="relu", bufs=2)
                        nc.vector.tensor_relu(out=relu_s[:], in_=q_s[:])
                        nc.tensor.matmul(t1ps_a[:], woff_t[hin][:, 0:128], relu_s[:],
                                         start=(hin == 0), stop=(hin == 7))
                        nc.tensor.matmul(t1ps_b[:], woff_t[hin][:, 128:256], relu_s[:],
                                         start=(hin == 0), stop=(hin == 7))
                    t1a = p1.tile([128, 512], dt.float32r, tag="t1a", bufs=2)
                    t1b = p1.tile([128, 512], dt.float32r, tag="t1b", bufs=2)
                    nc.vector.tensor_copy(out=t1a[:], in_=t1ps_a[:])
                    nc.vector.tensor_copy(out=t1b[:], in_=t1ps_b[:])
                    zps = ps1.tile([1, 512], dt.float32, tag="zps")
                    nc.tensor.matmul(zps[:], wlin_t[0][:], t1a[:], start=True, stop=False)
                    nc.tensor.matmul(zps[:], wlin_t[1][:], t1b[:], start=False, stop=True)
                    nc.scalar.activation(sig_row[:, s * 512:(s + 1) * 512], zps[:],
                                         AF.Sigmoid, bias=blin2_t[:], scale=1.0)

                nc.vector.scalar_tensor_tensor(sig_row[:], sig_row[:], float(BIG * L),
                                               iotab_t[:], op0=Alu.mult, op1=Alu.add)
                nc.gpsimd.partition_broadcast(wbig[:], sig_row[:], channels=128)

            # ================= phase 3: V projection =================
            with tc.tile_pool(name="ps3", bufs=3, space="PSUM") as ps3:
                for lb in range(16):
                    for h in range(2):
                        vps = ps3.tile([128, 512], dt.float32, tag="vps")
                        for hin in range(8):
                            nc.tensor.matmul(vps[:],
                                             qtb[hin][:, lb * 128:(lb + 1) * 128],
                                             wv_t[hin][:, h * 512:(h + 1) * 512],
                                             start=(hin == 0), stop=(hin == 7))
                        nc.vector.tensor_tensor(
                            out=VT[lb][:, h * 512:(h + 1) * 512], in0=vps[:],
                            in1=bvb[:, h * 512:(h + 1) * 512], op=Alu.add)

            # ================= phase 4: attention =================
            with tc.tile_pool(name="p4", bufs=1) as p4, \
                 tc.tile_pool(name="pss", bufs=3, space="PSUM") as pss, \
                 tc.tile_pool(name="psd", bufs=1, space="PSUM") as psd, \
                 tc.tile_pool(name="pso", bufs=4, space="PSUM") as pso:
                for g in range(G):
                    for s in range(4):
                        outps = [pso.tile([128, D], dt.float32, tag="outps", name="outps")
                                 for _ in range(4)]
                        denps = psd.tile([1, 512], dt.float32, tag="dps")
                        def consume(kb, pt, w):
                            nc.tensor.matmul(denps[:, :w], ones_t[:], pt[:, :w],
                                             start=(kb == 15), stop=(kb == 4 * s),
                                             skip_group_check=True)
                            for j in range(4):
                                if 4 * s + j <= kb:
                                    nc.tensor.matmul(
                                        outps[j][:],
                                        pt[:, j * 128:(j + 1) * 128],
                                        VT[kb][:, g * D:(g + 1) * D],
                                        start=(kb == 15), stop=(kb == 4 * s + j))

                        pending = None
                        for kb in range(15, 4 * s - 1, -1):
                            w = 512 if kb >= 4 * s + 3 else (kb - 4 * s + 1) * 128
                            q0 = s * 512
                            sps = pss.tile([128, 512], dt.float32, tag="sps")
                            nc.tensor.matmul(sps[:, :w],
                                             KT[g][0][:, kb * 128:(kb + 1) * 128],
                                             QT[g][0][:, q0:q0 + w],
                                             start=True, stop=False)
                            nc.tensor.matmul(sps[:, :w],
                                             KT[g][1][:, kb * 128:(kb + 1) * 128],
                                             QT[g][1][:, q0:q0 + w],
                                             start=False, stop=True)
                            if pending is not None:
                                consume(*pending)
                            z = p4.tile([128, 512], dt.float32, tag="z", bufs=4)
                            nc.vector.scalar_tensor_tensor(
                                z[:, :w], wbig[:, q0:q0 + w],
                                kvec_t[:, g * 16 + kb:g * 16 + kb + 1], sps[:, :w],
                                op0=Alu.subtract, op1=Alu.min)
                            if kb <= 4 * s + 3:
                                nc.vector.tensor_tensor(out=z[:, w - 128:w],
                                                        in0=z[:, w - 128:w],
                                                        in1=dt_t[:], op=Alu.min)
                            pt = p4.tile([128, 512], dt.bfloat16, tag="pt", bufs=4)
                            nc.scalar.activation(pt[:, :w], z[:, :w], AF.Exp,
                                                 scale=SCALE2)
                            pending = (kb, pt, w)
                        consume(*pending)
                        oraw = []
                        for j in range(4):
                            t = p4.tile([128, D], dt.float32, tag=f"oraw{j}",
                                        bufs=2, name=f"oraw{j}")
                            nc.vector.tensor_copy(out=t[:], in_=outps[j][:])
                            oraw.append(t)
                        row = g * 4 + s
                        den_sb = p4.tile([1, 512], dt.float32, tag="densb",
                                         bufs=2, name="densb")
                        nc.vector.tensor_copy(out=den_sb[:], in_=denps[:])
                        nc.sync.dma_start(out=dend[row:row + 1, :], in_=den_sb[:])
                        dentr = p4.tile([128, 4], dt.float32, tag="dentr",
                                        bufs=2, name="dentr")
                        nc.sync.dma_start(
                            out=dentr[:],
                            in_=dend[row:row + 1, :].rearrange(
                                "o (c p) -> (o p) c", p=128))
                        rden = p4.tile([128, 4], dt.float32, tag="rden",
                                       bufs=2, name="rden")
                        nc.vector.reciprocal(out=rden[:], in_=dentr[:])
                        for j in range(4):
                            c = s * 4 + j
                            outn = p4.tile([128, D], dt.float32, tag="outn", bufs=3)
                            nc.vector.tensor_scalar(out=outn[:], in0=oraw[j][:],
                                                    scalar1=rden[:, j:j + 1],
                                                    scalar2=None, op0=Alu.mult)
                            nc.sync.dma_start(
                                out=out[c * 128:(c + 1) * 128, g * D:(g + 1) * D],
                                in_=outn[:])

    nc.finalize()
    return nc


def _prep_shared(qkv_w, qkv_b, off_w, lin2_w, lin2_b):
    f32 = np.float32
    bf = ml_dtypes.bfloat16
    qkv_wT = np.ascontiguousarray(qkv_w.T, dtype=f32)          # [H, 3H]
    shared = {
        "wqk": qkv_wT[:, :2 * H].astype(bf),
        "wv": np.ascontiguousarray(qkv_wT[:, 2 * H:]).astype(bf),
        "woff": np.ascontiguousarray(off_w.T, dtype=f32),
        "wlin2": np.ascontiguousarray(lin2_w.T, dtype=f32),
        "bqk": np.ascontiguousarray(
            qkv_b[:2 * H].reshape(16, 128).T, dtype=f32),
        "bv": np.ascontiguousarray(
            np.broadcast_to(qkv_b[2 * H:][None], (128, H))).astype(bf),
        "blin2": np.asarray(lin2_b, dtype=f32).reshape(1, 1),
        "iotab": (BIG * np.arange(L, dtype=np.float64)).astype(f32)[None],
    }
    p = np.arange(128, dtype=np.float64)[:, None]
    cols = []
    for g in range(G):
        for kb in range(16):
            cols.append(BIG * (kb * 128 + p - WS[g]))
    shared["kvec"] = np.concatenate(cols, axis=1).astype(f32)
    pi = np.arange(128)[:, None]
    fi = np.arange(128)[None, :]
    shared["dtile"] = np.where(pi >= fi, 1e6, -1e6).astype(f32)
    return shared


def kernel(query, key_in, value, qkv_w, qkv_b, off_w, lin2_w, lin2_b,
           _trace=False, _tmpdir=None):
    global _BUILT
    query = np.asarray(query, dtype=np.float32)
    shared = _prep_shared(np.asarray(qkv_w, np.float32),
                          np.asarray(qkv_b, np.float32),
                          np.asarray(off_w, np.float32),
                          np.asarray(lin2_w, np.float32),
                          np.asarray(lin2_b, np.float32))
    in_maps = []
    for b in range(NCORES):
        qT = np.ascontiguousarray(query[b].T)                  # [H, L]
        m = dict(shared)
        m["qt32"] = qT
        m["qtbf"] = qT.astype(ml_dtypes.bfloat16)
        in_maps.append(m)

    if _BUILT is None:
        _BUILT = build_nc()
    kw = {}
    if _trace:
        kw = dict(trace=True, tmpdir=_tmpdir)
    res = run_bass_kernel_spmd(_BUILT, in_maps, core_ids=list(range(NCORES)), **kw)
    out = np.stack([res.results[b]["out"] for b in range(NCORES)], axis=0)
    if _trace:
        return out, res
    return out


if __name__ == "__main__":
    rng = np.random.default_rng(0)
    ins = {
        "query": rng.standard_normal((B, L, H)).astype(np.float32),
        "key_in": rng.standard_normal((B, L, H)).astype(np.float32),
        "value": rng.standard_normal((B, L, H)).astype(np.float32),
        "qkv_w": (rng.standard_normal((3 * H, H)) * 0.02).astype(np.float32),
        "qkv_b": np.zeros(3 * H, np.float32),
        "off_w": (rng.standard_normal((D1, H)) * 0.02).astype(np.float32),
        "lin2_w": (rng.standard_normal((1, D1)) * 0.02).astype(np.float32),
        "lin2_b": np.zeros(1, np.float32),
    }
    o = kernel(**ins)
    print("out", o.shape, o.dtype, np.abs(o).mean())


# revision 27
# speedup vs baseline: 1.2368x; 1.2368x over previous
"""Trainium2 Bass kernel for nn_MultiHeadAttention_28028956574019.

Sparse windowed multi-head attention, G=4 window groups, learned per-row
window offset. Data-parallel over batch: 8 NeuronCores, one batch element
per core.

Per-core device program (L=2048, H=1024, d=256 per group):
  phase 1: learned-offset path dx = L*sigmoid(lin2(relu(q) @ off_w.T))
           computed in fp32r; mask row W = BIG*(q_idx + dx) broadcast to
           [128, 2048] via gpsimd partition_broadcast.
  phase 2: Q/K projection (bf16): Q_T/K_T stored [h, l] (transposed).
  phase 3: V projection (bf16): V stored [l, h] (natural).
  phase 4: per group, per 512-wide q-strip, k-blocks descending 15..4s:
           S_T[k,q] = K_T.T@Q_T (PSUM f32), masked via
           Z = min(S, W - BIG*(k - ws)) [one DVE scalar_tensor_tensor]
           + diagonal lower-bound tile min, p = exp(SCALE2*Z) -> bf16,
           denominator via ones-matmul (PSUM accumulate), out += p.T@V.
           Normalization deferred: denom row DMA round-trips through DRAM
           to per-partition layout, out_raw * recip(denom).

All shapes hardcoded for the fixed problem size. The harness calls
kernel(**inputs) with the full (unsharded) inputs.
"""

import sys

if "/opt/trn_rl_repo" not in sys.path:
    sys.path.insert(0, "/opt/trn_rl_repo")

import numpy as np
import ml_dtypes

import concourse.bass as bass  # noqa: F401  (bass must import before bacc)
from concourse import bacc
import concourse.mybir as mybir
from concourse.tile import TileContext
from concourse.bass_utils import run_bass_kernel_spmd

dt = mybir.dt
AF = mybir.ActivationFunctionType
Alu = mybir.AluOpType

B, L, H = 8, 2048, 1024
G, D = 4, 256          # groups, per-group head dim
D1 = 256               # learned-offset hidden dim
WS = [4, 16, 64, 256]
BIG = 1.0e7
SCALE2 = 2.0 / float(np.sqrt(L))   # masked_fill+add doubles unmasked scores
NCORES = 8

_BUILT = None


def build_nc(trace_friendly=False):
    nc = bacc.Bacc("TRN2", target_bir_lowering=False, debug=False)

    # ---- I/O ----
    qt32 = nc.declare_dram_parameter("qt32", [H, L], dt.float32, isOutput=False)
    qtbf = nc.declare_dram_parameter("qtbf", [H, L], dt.bfloat16, isOutput=False)
    wqk = nc.declare_dram_parameter("wqk", [H, 2 * H], dt.bfloat16, isOutput=False)
    wv = nc.declare_dram_parameter("wv", [H, H], dt.bfloat16, isOutput=False)
    woff = nc.declare_dram_parameter("woff", [H, D1], dt.float32, isOutput=False)
    wlin2 = nc.declare_dram_parameter("wlin2", [D1, 1], dt.float32, isOutput=False)
    bqk = nc.declare_dram_parameter("bqk", [128, 16], dt.float32, isOutput=False)
    bv = nc.declare_dram_parameter("bv", [128, H], dt.bfloat16, isOutput=False)
    blin2 = nc.declare_dram_parameter("blin2", [1, 1], dt.float32, isOutput=False)
    dtile = nc.declare_dram_parameter("dtile", [128, 128], dt.float32, isOutput=False)
    iotab = nc.declare_dram_parameter("iotab", [1, L], dt.float32, isOutput=False)
    kvec = nc.declare_dram_parameter("kvec", [128, 64], dt.float32, isOutput=False)
    out = nc.declare_dram_parameter("out", [L, H], dt.float32, isOutput=True)


    with TileContext(nc) as tc:
        with tc.tile_pool(name="persist", bufs=1) as pp:
            bqk_t = pp.tile([128, 16], dt.float32, name="bqk_t")
            nc.gpsimd.dma_start(out=bqk_t[:], in_=bqk[:])
            # bf16 query (transposed), resident for phases 2-3
            qtb = []
            for i in range(8):
                t = pp.tile([128, L], dt.bfloat16, name=f"qtb{i}")
                nc.gpsimd.dma_start(out=t[:], in_=qtbf[i * 128:(i + 1) * 128, :])
                qtb.append(t)
            # ---- consts ----
            dt_t = pp.tile([128, 128], dt.float32, name="dt_t")
            nc.gpsimd.dma_start(out=dt_t[:], in_=dtile[:])
            kvec_t = pp.tile([128, 64], dt.float32, name="kvec_t")
            nc.gpsimd.dma_start(out=kvec_t[:], in_=kvec[:])
            blin2_t = pp.tile([1, 1], dt.float32, name="blin2_t")
            nc.gpsimd.dma_start(out=blin2_t[:], in_=blin2[:])
            bvb = pp.tile([128, H], dt.bfloat16, name="bvb")
            nc.gpsimd.dma_start(out=bvb[:], in_=bv[:])
            ones_t = pp.tile([128, 1], dt.bfloat16, name="ones_t")
            nc.vector.memset(ones_t[:], 1.0)
            one1 = pp.tile([1, 1], dt.float32, name="one1")
            nc.vector.memset(one1[:], 1.0)
            wbig = pp.tile([128, L], dt.float32, name="wbig")

            # bf16 V-projection weights, resident
            wv_t = []
            for i in range(8):
                t = pp.tile([128, H], dt.bfloat16, name=f"wv{i}")
                nc.gpsimd.dma_start(out=t[:], in_=wv[i * 128:(i + 1) * 128, :])
                wv_t.append(t)

            # persistent Q_T / K_T / V
            QT = [[pp.tile([128, L], dt.bfloat16, name=f"QT{g}{h}", tag=f"QT{g}{h}") for h in range(2)]
                  for g in range(G)]
            KT = [[pp.tile([128, L], dt.bfloat16, name=f"KT{g}{h}", tag=f"KT{g}{h}") for h in range(2)]
                  for g in range(G)]
            VT = []
            for lb in range(16):
                t = pp.tile([128, 4 * (D + 1)], dt.bfloat16, name=f"VT{lb}",
                            tag=f"VT{lb}")
                nc.vector.memset(t[:, D::D + 1], 1.0)
                VT.append(t)

            # ================= phase 2: Q/K projection =================
            with tc.tile_pool(name="p2", bufs=1) as p2, \
                 tc.tile_pool(name="ps2", bufs=3, space="PSUM") as ps2:
                for hb in range(16):
                    wt = []
                    for hin in range(8):
                        t = p2.tile([128, 128], dt.bfloat16, tag="wqk", bufs=16)
                        nc.sync.dma_start(
                            out=t[:],
                            in_=wqk[hin * 128:(hin + 1) * 128, hb * 128:(hb + 1) * 128])
                        wt.append(t)
                    g, h = (hb % 8) // 2, hb % 2
                    dest = QT[g][h] if hb < 8 else KT[g][h]
                    for s in range(4):
                        pps = ps2.tile([128, 512], dt.float32, tag="qkps")
                        for hin in range(8):
                            nc.tensor.matmul(pps[:], wt[hin][:],
                                             qtb[hin][:, s * 512:(s + 1) * 512],
                                             start=(hin == 0), stop=(hin == 7))
                        nc.scalar.activation(dest[:, s * 512:(s + 1) * 512], pps[:],
                                             AF.Identity, bias=bqk_t[:, hb:hb + 1],
                                             scale=1.0)

            # ================= phase 1: dx path =================
            with tc.tile_pool(name="p1", bufs=1) as p1, \
                 tc.tile_pool(name="ps1", bufs=2, space="PSUM") as ps1:
                woff_t = []
                for i in range(8):
                    t = p1.tile([128, D1], dt.float32r, name=f"woff{i}")
                    nc.gpsimd.dma_start(out=t[:],
                                      in_=woff[i * 128:(i + 1) * 128, :].bitcast(dt.float32r))
                    woff_t.append(t)
                wlin_t = []
                for i in range(2):
                    t = p1.tile([128, 1], dt.float32r, name=f"wlin{i}")
                    nc.gpsimd.dma_start(out=t[:],
                                      in_=wlin2[i * 128:(i + 1) * 128, :].bitcast(dt.float32r))
                    wlin_t.append(t)
                sig_row = p1.tile([1, L], dt.float32, name="sig_row")
                iotab_t = p1.tile([1, L], dt.float32, name="iotab_t")
                nc.gpsimd.dma_start(out=iotab_t[:], in_=iotab[:])

                qs_tiles = {}
                for s in range(4):
                    for hin in range(8):
                        q_s = p1.tile([128, 512], dt.float32, tag="qs", bufs=6,
                                      name="q_s")
                        nc.gpsimd.dma_start(
                            out=q_s[:],
                            in_=qt32[hin * 128:(hin + 1) * 128, s * 512:(s + 1) * 512])
                        qs_tiles[s, hin] = q_s
                def z_stage(s, t1a, t1b):
                    zps = ps1.tile([1, 512], dt.float32, tag="zps")
                    nc.tensor.matmul(zps[:], wlin_t[0][:], t1a[:], start=True, stop=False)
                    nc.tensor.matmul(zps[:], wlin_t[1][:], t1b[:], start=False, stop=True)
                    nc.scalar.activation(sig_row[:, s * 512:(s + 1) * 512], zps[:],
                                         AF.Sigmoid, bias=blin2_t[:], scale=1.0)

                zpend = None
                for s in range(4):
                    t1ps_a = ps1.tile([128, 512], dt.float32, tag="t1psa", bufs=2)
                    t1ps_b = ps1.tile([128, 512], dt.float32, tag="t1psb", bufs=2)
                    for hin in range(8):
                        q_s = qs_tiles[s, hin]
                        relu_s = p1.tile([128, 512], dt.float32r, tag="relu", bufs=2)
                        nc.vector.tensor_relu(out=relu_s[:], in_=q_s[:])
                        nc.tensor.matmul(t1ps_a[:], woff_t[hin][:, 0:128], relu_s[:],
                                         start=(hin == 0), stop=(hin == 7))
                        nc.tensor.matmul(t1ps_b[:], woff_t[hin][:, 128:256], relu_s[:],
                                         start=(hin == 0), stop=(hin == 7))
                        if hin == 3 and zpend is not None:
                            z_stage(*zpend)
                            zpend = None
                    t1a = p1.tile([128, 512], dt.float32r, tag="t1a", bufs=2)
                    t1b = p1.tile([128, 512], dt.float32r, tag="t1b", bufs=2)
                    nc.vector.tensor_copy(out=t1a[:], in_=t1ps_a[:])
                    nc.vector.tensor_copy(out=t1b[:], in_=t1ps_b[:])
                    zpend = (s, t1a, t1b)
                z_stage(*zpend)

                nc.vector.scalar_tensor_tensor(sig_row[:], sig_row[:], float(BIG * L),
                                               iotab_t[:], op0=Alu.mult, op1=Alu.add)
                nc.gpsimd.partition_broadcast(wbig[:], sig_row[:], channels=128)

            # ================= phase 3: V projection =================
            with tc.tile_pool(name="ps3", bufs=3, space="PSUM") as ps3:
                for lb in range(16):
                    for h in range(2):
                        vps = ps3.tile([128, 512], dt.float32, tag="vps")
                        for hin in range(8):
                            nc.tensor.matmul(vps[:],
                                             qtb[hin][:, lb * 128:(lb + 1) * 128],
                                             wv_t[hin][:, h * 512:(h + 1) * 512],
                                             start=(hin == 0), stop=(hin == 7))
                        for gg in range(2):
                            g2 = h * 2 + gg
                            nc.vector.tensor_tensor(
                                out=VT[lb][:, g2 * (D + 1):g2 * (D + 1) + D],
                                in0=vps[:, gg * D:(gg + 1) * D],
                                in1=bvb[:, g2 * D:(g2 + 1) * D], op=Alu.add)

            # ================= phase 4: attention =================
            with tc.tile_pool(name="p4", bufs=1) as p4, \
                 tc.tile_pool(name="pss", bufs=4, space="PSUM") as pss, \
                 tc.tile_pool(name="pso", bufs=4, space="PSUM") as pso:
                for g in range(G):
                    for s in range(4):
                        outps = [pso.tile([128, D + 1], dt.float32, tag="outps",
                                          name="outps")
                                 for _ in range(4)]
                        def consume(kb, pt, w):
                            for j in range(4):
                                if 4 * s + j <= kb:
                                    nc.tensor.matmul(
                                        outps[j][:],
                                        pt[:, j * 128:(j + 1) * 128],
                                        VT[kb][:, g * (D + 1):(g + 1) * (D + 1)],
                                        start=(kb == 15), stop=(kb == 4 * s + j))

                        pending = []
                        for kb in range(15, 4 * s - 1, -1):
                            w = 512 if kb >= 4 * s + 3 else (kb - 4 * s + 1) * 128
                            q0 = s * 512
                            sps = pss.tile([128, 512], dt.float32, tag="sps")
                            nc.tensor.matmul(sps[:, :w],
                                             KT[g][0][:, kb * 128:(kb + 1) * 128],
                                             QT[g][0][:, q0:q0 + w],
                                             start=True, stop=False)
                            nc.tensor.matmul(sps[:, :w],
                                             KT[g][1][:, kb * 128:(kb + 1) * 128],
                                             QT[g][1][:, q0:q0 + w],
                                             start=False, stop=True)
                            if len(pending) >= 3:
                                consume(*pending.pop(0))
                            z = p4.tile([128, 512], dt.float32, tag="z", bufs=4)
                            nc.vector.scalar_tensor_tensor(
                                z[:, :w], wbig[:, q0:q0 + w],
                                kvec_t[:, g * 16 + kb:g * 16 + kb + 1], sps[:, :w],
                                op0=Alu.subtract, op1=Alu.min)
                            if kb <= 4 * s + 3:
                                nc.vector.tensor_tensor(out=z[:, w - 128:w],
                                                        in0=z[:, w - 128:w],
                                                        in1=dt_t[:], op=Alu.min)
                            pt = p4.tile([128, 512], dt.bfloat16, tag="pt", bufs=4)
                            nc.scalar.activation(pt[:, :w], z[:, :w], AF.Exp,
                                                 scale=SCALE2)
                            pending.append((kb, pt, w))
                        for it in pending:
                            consume(*it)
                        for j in range(4):
                            c = s * 4 + j
                            rden = p4.tile([128, 1], dt.float32, tag="rden",
                                           bufs=4, name="rden")
                            nc.vector.reciprocal(out=rden[:],
                                                 in_=outps[j][:, D:D + 1])
                            outn = p4.tile([128, D], dt.float32, tag="outn", bufs=4)
                            nc.vector.tensor_scalar(out=outn[:],
                                                    in0=outps[j][:, :D],
                                                    scalar1=rden[:], scalar2=None,
                                                    op0=Alu.mult)
                            nc.sync.dma_start(
                                out=out[c * 128:(c + 1) * 128, g * D:(g + 1) * D],
                                in_=outn[:])

    nc.finalize()
    return nc


def _prep_shared(qkv_w, qkv_b, off_w, lin2_w, lin2_b):
    f32 = np.float32
    bf = ml_dtypes.bfloat16
    qkv_wT = np.ascontiguousarray(qkv_w.T, dtype=f32)          # [H, 3H]
    shared = {
        "wqk": qkv_wT[:, :2 * H].astype(bf),
        "wv": np.ascontiguousarray(qkv_wT[:, 2 * H:]).astype(bf),
        "woff": np.ascontiguousarray(off_w.T, dtype=f32),
        "wlin2": np.ascontiguousarray(lin2_w.T, dtype=f32),
        "bqk": np.ascontiguousarray(
            qkv_b[:2 * H].reshape(16, 128).T, dtype=f32),
        "bv": np.ascontiguousarray(
            np.broadcast_to(qkv_b[2 * H:][None], (128, H))).astype(bf),
        "blin2": np.asarray(lin2_b, dtype=f32).reshape(1, 1),
        "iotab": (BIG * np.arange(L, dtype=np.float64)).astype(f32)[None],
    }
    p = np.arange(128, dtype=np.float64)[:, None]
    cols = []
    for g in range(G):
        for kb in range(16):
            cols.append(BIG * (kb * 128 + p - WS[g]))
    shared["kvec"] = np.concatenate(cols, axis=1).astype(f32)
    pi = np.arange(128)[:, None]
    fi = np.arange(128)[None, :]
    shared["dtile"] = np.where(pi >= fi, 1e6, -1e6).astype(f32)
    return shared


def kernel(query, key_in, value, qkv_w, qkv_b, off_w, lin2_w, lin2_b,
           _trace=False, _tmpdir=None):
    global _BUILT
    query = np.asarray(query, dtype=np.float32)
    shared = _prep_shared(np.asarray(qkv_w, np.float32),
                          np.asarray(qkv_b, np.float32),
                          np.asarray(off_w, np.float32),
                          np.asarray(lin2_w, np.float32),
                          np.asarray(lin2_b, np.float32))
    in_maps = []
    for b in range(NCORES):
        qT = np.ascontiguousarray(query[b].T)                  # [H, L]
        m = dict(shared)
        m["qt32"] = qT
        m["qtbf"] = qT.astype(ml_dtypes.bfloat16)
        in_maps.append(m)

    if _BUILT is None:
        _BUILT = build_nc()
    kw = {}
    if _trace:
        kw = dict(trace=True, tmpdir=_tmpdir)
    res = run_bass_kernel_spmd(_BUILT, in_maps, core_ids=list(range(NCORES)), **kw)
    out = np.stack([res.results[b]["out"] for b in range(NCORES)], axis=0)
    if _trace:
        return out, res
    return out


if __name__ == "__main__":
    rng = np.random.default_rng(0)
    ins = {
        "query": rng.standard_normal((B, L, H)).astype(np.float32),
        "key_in": rng.standard_normal((B, L, H)).astype(np.float32),
        "value": rng.standard_normal((B, L, H)).astype(np.float32),
        "qkv_w": (rng.standard_normal((3 * H, H)) * 0.02).astype(np.float32),
        "qkv_b": np.zeros(3 * H, np.float32),
        "off_w": (rng.standard_normal((D1, H)) * 0.02).astype(np.float32),
        "lin2_w": (rng.standard_normal((1, D1)) * 0.02).astype(np.float32),
        "lin2_b": np.zeros(1, np.float32),
    }
    o = kernel(**ins)
    print("out", o.shape, o.dtype, np.abs(o).mean())


# revision 28
# speedup vs baseline: 1.2447x; 1.0064x over previous
"""Trainium2 Bass kernel for nn_MultiHeadAttention_28028956574019.

Sparse windowed multi-head attention, G=4 window groups, learned per-row
window offset. Data-parallel over batch: 8 NeuronCores, one batch element
per core.

Per-core device program (L=2048, H=1024, d=256 per group):
  phase 1: learned-offset path dx = L*sigmoid(lin2(relu(q) @ off_w.T))
           computed in fp32r; mask row W = BIG*(q_idx + dx) broadcast to
           [128, 2048] via gpsimd partition_broadcast.
  phase 2: Q/K projection (bf16): Q_T/K_T stored [h, l] (transposed).
  phase 3: V projection (bf16): V stored [l, h] (natural).
  phase 4: per group, per 512-wide q-strip, k-blocks descending 15..4s:
           S_T[k,q] = K_T.T@Q_T (PSUM f32), masked via
           Z = min(S, W - BIG*(k - ws)) [one DVE scalar_tensor_tensor]
           + diagonal lower-bound tile min, p = exp(SCALE2*Z) -> bf16,
           denominator via ones-matmul (PSUM accumulate), out += p.T@V.
           Normalization deferred: denom row DMA round-trips through DRAM
           to per-partition layout, out_raw * recip(denom).

All shapes hardcoded for the fixed problem size. The harness calls
kernel(**inputs) with the full (unsharded) inputs.
"""

import sys

if "/opt/trn_rl_repo" not in sys.path:
    sys.path.insert(0, "/opt/trn_rl_repo")

import numpy as np
import ml_dtypes

import concourse.bass as bass  # noqa: F401  (bass must import before bacc)
from concourse import bacc
import concourse.mybir as mybir
from concourse.tile import TileContext
from concourse.bass_utils import run_bass_kernel_spmd

dt = mybir.dt
AF = mybir.ActivationFunctionType
Alu = mybir.AluOpType

B, L, H = 8, 2048, 1024
G, D = 4, 256          # groups, per-group head dim
D1 = 256               # learned-offset hidden dim
WS = [4, 16, 64, 256]
BIG = 1.0e7
SCALE2 = 2.0 / float(np.sqrt(L))   # masked_fill+add doubles unmasked scores
NCORES = 8

_BUILT = None


def build_nc(trace_friendly=False):
    nc = bacc.Bacc("TRN2", target_bir_lowering=False, debug=False)

    # ---- I/O ----
    qt32 = nc.declare_dram_parameter("qt32", [H, L], dt.float32, isOutput=False)
    qtbf = nc.declare_dram_parameter("qtbf", [H, L], dt.bfloat16, isOutput=False)
    wqk = nc.declare_dram_parameter("wqk", [H, 2 * H], dt.bfloat16, isOutput=False)
    wv = nc.declare_dram_parameter("wv", [H, H], dt.bfloat16, isOutput=False)
    woff = nc.declare_dram_parameter("woff", [H, D1], dt.float32, isOutput=False)
    wlin2 = nc.declare_dram_parameter("wlin2", [D1, 1], dt.float32, isOutput=False)
    bqk = nc.declare_dram_parameter("bqk", [128, 16], dt.float32, isOutput=False)
    bv = nc.declare_dram_parameter("bv", [128, H], dt.bfloat16, isOutput=False)
    blin2 = nc.declare_dram_parameter("blin2", [1, 1], dt.float32, isOutput=False)
    dtile = nc.declare_dram_parameter("dtile", [128, 128], dt.float32, isOutput=False)
    iotab = nc.declare_dram_parameter("iotab", [1, L], dt.float32, isOutput=False)
    kvec = nc.declare_dram_parameter("kvec", [128, 64], dt.float32, isOutput=False)
    out = nc.declare_dram_parameter("out", [L, H], dt.float32, isOutput=True)


    with TileContext(nc) as tc:
        with tc.tile_pool(name="persist", bufs=1) as pp:
            bqk_t = pp.tile([128, 16], dt.float32, name="bqk_t")
            nc.gpsimd.dma_start(out=bqk_t[:], in_=bqk[:])
            # bf16 query (transposed), resident for phases 2-3
            qtb = []
            for i in range(8):
                t = pp.tile([128, L], dt.bfloat16, name=f"qtb{i}")
                nc.gpsimd.dma_start(out=t[:], in_=qtbf[i * 128:(i + 1) * 128, :])
                qtb.append(t)
            # ---- consts ----
            dt_t = pp.tile([128, 128], dt.float32, name="dt_t")
            nc.gpsimd.dma_start(out=dt_t[:], in_=dtile[:])
            kvec_t = pp.tile([128, 64], dt.float32, name="kvec_t")
            nc.gpsimd.dma_start(out=kvec_t[:], in_=kvec[:])
            blin2_t = pp.tile([1, 1], dt.float32, name="blin2_t")
            nc.gpsimd.dma_start(out=blin2_t[:], in_=blin2[:])
            bvb = pp.tile([128, H], dt.bfloat16, name="bvb")
            nc.gpsimd.dma_start(out=bvb[:], in_=bv[:])
            ones_t = pp.tile([128, 1], dt.bfloat16, name="ones_t")
            nc.vector.memset(ones_t[:], 1.0)
            one1 = pp.tile([1, 1], dt.float32, name="one1")
            nc.vector.memset(one1[:], 1.0)
            wbig = pp.tile([128, L], dt.float32, name="wbig")

            # bf16 V-projection weights, resident
            wv_t = []
            for i in range(8):
                t = pp.tile([128, H], dt.bfloat16, name=f"wv{i}")
                nc.gpsimd.dma_start(out=t[:], in_=wv[i * 128:(i + 1) * 128, :])
                wv_t.append(t)

            # persistent Q_T / K_T / V
            QT = [[pp.tile([128, L], dt.bfloat16, name=f"QT{g}{h}", tag=f"QT{g}{h}") for h in range(2)]
                  for g in range(G)]
            KT = [[pp.tile([128, L], dt.bfloat16, name=f"KT{g}{h}", tag=f"KT{g}{h}") for h in range(2)]
                  for g in range(G)]
            VT = []
            for lb in range(16):
                t = pp.tile([128, 4 * (D + 1)], dt.bfloat16, name=f"VT{lb}",
                            tag=f"VT{lb}")
                nc.vector.memset(t[:, D::D + 1], 1.0)
                VT.append(t)

            # ================= phase 2: Q/K projection =================
            with tc.tile_pool(name="p2", bufs=1) as p2, \
                 tc.tile_pool(name="ps2", bufs=3, space="PSUM") as ps2:
                for hb in range(16):
                    wt = []
                    for hin in range(8):
                        t = p2.tile([128, 128], dt.bfloat16, tag="wqk", bufs=16)
                        nc.sync.dma_start(
                            out=t[:],
                            in_=wqk[hin * 128:(hin + 1) * 128, hb * 128:(hb + 1) * 128])
                        wt.append(t)
                    g, h = (hb % 8) // 2, hb % 2
                    dest = QT[g][h] if hb < 8 else KT[g][h]
                    for s in range(4):
                        pps = ps2.tile([128, 512], dt.float32, tag="qkps")
                        for hin in range(8):
                            nc.tensor.matmul(pps[:], wt[hin][:],
                                             qtb[hin][:, s * 512:(s + 1) * 512],
                                             start=(hin == 0), stop=(hin == 7))
                        nc.scalar.activation(dest[:, s * 512:(s + 1) * 512], pps[:],
                                             AF.Identity, bias=bqk_t[:, hb:hb + 1],
                                             scale=1.0)

            # ================= phase 1: dx path =================
            with tc.tile_pool(name="p1", bufs=1) as p1, \
                 tc.tile_pool(name="ps1", bufs=2, space="PSUM") as ps1:
                woff_t = []
                for i in range(8):
                    t = p1.tile([128, D1], dt.float32r, name=f"woff{i}")
                    nc.gpsimd.dma_start(out=t[:],
                                      in_=woff[i * 128:(i + 1) * 128, :].bitcast(dt.float32r))
                    woff_t.append(t)
                wlin_t = []
                for i in range(2):
                    t = p1.tile([128, 1], dt.float32r, name=f"wlin{i}")
                    nc.gpsimd.dma_start(out=t[:],
                                      in_=wlin2[i * 128:(i + 1) * 128, :].bitcast(dt.float32r))
                    wlin_t.append(t)
                sig_row = p1.tile([1, L], dt.float32, name="sig_row")
                iotab_t = p1.tile([1, L], dt.float32, name="iotab_t")
                nc.gpsimd.dma_start(out=iotab_t[:], in_=iotab[:])

                qs_tiles = {}
                for s in range(4):
                    for hin in range(8):
                        q_s = p1.tile([128, 512], dt.float32, tag="qs", bufs=6,
                                      name="q_s")
                        nc.gpsimd.dma_start(
                            out=q_s[:],
                            in_=qt32[hin * 128:(hin + 1) * 128, s * 512:(s + 1) * 512])
                        qs_tiles[s, hin] = q_s
                def z_stage(s, t1a, t1b):
                    zps = ps1.tile([1, 512], dt.float32, tag="zps")
                    nc.tensor.matmul(zps[:], wlin_t[0][:], t1a[:], start=True, stop=False)
                    nc.tensor.matmul(zps[:], wlin_t[1][:], t1b[:], start=False, stop=True)
                    nc.scalar.activation(sig_row[:, s * 512:(s + 1) * 512], zps[:],
                                         AF.Sigmoid, bias=blin2_t[:], scale=1.0)

                zpend = None
                for s in range(4):
                    t1ps_a = ps1.tile([128, 512], dt.float32, tag="t1psa", bufs=2)
                    t1ps_b = ps1.tile([128, 512], dt.float32, tag="t1psb", bufs=2)
                    for hin in range(8):
                        q_s = qs_tiles[s, hin]
                        relu_s = p1.tile([128, 512], dt.float32r, tag="relu", bufs=2)
                        nc.vector.tensor_relu(out=relu_s[:], in_=q_s[:])
                        nc.tensor.matmul(t1ps_a[:], woff_t[hin][:, 0:128], relu_s[:],
                                         start=(hin == 0), stop=(hin == 7))
                        nc.tensor.matmul(t1ps_b[:], woff_t[hin][:, 128:256], relu_s[:],
                                         start=(hin == 0), stop=(hin == 7))
                        if hin == 3 and zpend is not None:
                            z_stage(*zpend)
                            zpend = None
                    t1a = p1.tile([128, 512], dt.float32r, tag="t1a", bufs=2)
                    t1b = p1.tile([128, 512], dt.float32r, tag="t1b", bufs=2)
                    nc.vector.tensor_copy(out=t1a[:], in_=t1ps_a[:])
                    nc.vector.tensor_copy(out=t1b[:], in_=t1ps_b[:])
                    zpend = (s, t1a, t1b)
                z_stage(*zpend)

                nc.vector.scalar_tensor_tensor(sig_row[:], sig_row[:], float(BIG * L),
                                               iotab_t[:], op0=Alu.mult, op1=Alu.add)
                nc.gpsimd.partition_broadcast(wbig[:], sig_row[:], channels=128)

            # ================= phase 3: V projection =================
            with tc.tile_pool(name="ps3", bufs=3, space="PSUM") as ps3:
                for lb in range(16):
                    for h in range(2):
                        vps = ps3.tile([128, 512], dt.float32, tag="vps")
                        for hin in range(8):
                            nc.tensor.matmul(vps[:],
                                             qtb[hin][:, lb * 128:(lb + 1) * 128],
                                             wv_t[hin][:, h * 512:(h + 1) * 512],
                                             start=(hin == 0), stop=(hin == 7))
                        for gg in range(2):
                            g2 = h * 2 + gg
                            nc.vector.tensor_tensor(
                                out=VT[lb][:, g2 * (D + 1):g2 * (D + 1) + D],
                                in0=vps[:, gg * D:(gg + 1) * D],
                                in1=bvb[:, g2 * D:(g2 + 1) * D], op=Alu.add)

            # ================= phase 4: attention =================
            with tc.tile_pool(name="p4", bufs=1) as p4, \
                 tc.tile_pool(name="pss", bufs=4, space="PSUM") as pss, \
                 tc.tile_pool(name="pso", bufs=4, space="PSUM") as pso:
                for g in range(G):
                    for s in range(4):
                        outps = [pso.tile([128, D + 1], dt.float32, tag="outps",
                                          name="outps")
                                 for _ in range(4)]
                        def consume(kb, pt, w):
                            for j in range(4):
                                if 4 * s + j <= kb:
                                    nc.tensor.matmul(
                                        outps[j][:],
                                        pt[:, j * 128:(j + 1) * 128],
                                        VT[kb][:, g * (D + 1):(g + 1) * (D + 1)],
                                        start=(kb == 15), stop=(kb == 4 * s + j))

                        pending = []
                        for kb in range(15, 4 * s - 1, -1):
                            w = 512 if kb >= 4 * s + 3 else (kb - 4 * s + 1) * 128
                            q0 = s * 512
                            sps = pss.tile([128, 512], dt.float32, tag="sps")
                            nc.tensor.matmul(sps[:, :w],
                                             KT[g][0][:, kb * 128:(kb + 1) * 128],
                                             QT[g][0][:, q0:q0 + w],
                                             start=True, stop=False)
                            nc.tensor.matmul(sps[:, :w],
                                             KT[g][1][:, kb * 128:(kb + 1) * 128],
                                             QT[g][1][:, q0:q0 + w],
                                             start=False, stop=True)
                            if len(pending) >= 3:
                                consume(*pending.pop(0))
                            z = p4.tile([128, 512], dt.bfloat16, tag="z", bufs=4)
                            nc.vector.scalar_tensor_tensor(
                                z[:, :w], wbig[:, q0:q0 + w],
                                kvec_t[:, g * 16 + kb:g * 16 + kb + 1], sps[:, :w],
                                op0=Alu.subtract, op1=Alu.min)
                            if kb <= 4 * s + 3:
                                nc.vector.tensor_tensor(out=z[:, w - 128:w],
                                                        in0=z[:, w - 128:w],
                                                        in1=dt_t[:], op=Alu.min)
                            pt = p4.tile([128, 512], dt.bfloat16, tag="pt", bufs=4)
                            nc.scalar.activation(pt[:, :w], z[:, :w], AF.Exp,
                                                 scale=SCALE2)
                            pending.append((kb, pt, w))
                        for it in pending:
                            consume(*it)
                        for j in (3, 2, 1, 0):
                            c = s * 4 + j
                            rden = p4.tile([128, 1], dt.float32, tag="rden",
                                           bufs=4, name="rden")
                            nc.vector.reciprocal(out=rden[:],
                                                 in_=outps[j][:, D:D + 1])
                            outn = p4.tile([128, D], dt.float32, tag="outn", bufs=4)
                            nc.vector.tensor_scalar(out=outn[:],
                                                    in0=outps[j][:, :D],
                                                    scalar1=rden[:], scalar2=None,
                                                    op0=Alu.mult)
                            nc.sync.dma_start(
                                out=out[c * 128:(c + 1) * 128, g * D:(g + 1) * D],
                                in_=outn[:])

    nc.finalize()
    return nc


def _prep_shared(qkv_w, qkv_b, off_w, lin2_w, lin2_b):
    f32 = np.float32
    bf = ml_dtypes.bfloat16
    qkv_wT = np.ascontiguousarray(qkv_w.T, dtype=f32)          # [H, 3H]
    shared = {
        "wqk": qkv_wT[:, :2 * H].astype(bf),
        "wv": np.ascontiguousarray(qkv_wT[:, 2 * H:]).astype(bf),
        "woff": np.ascontiguousarray(off_w.T, dtype=f32),
        "wlin2": np.ascontiguousarray(lin2_w.T, dtype=f32),
        "bqk": np.ascontiguousarray(
            qkv_b[:2 * H].reshape(16, 128).T, dtype=f32),
        "bv": np.ascontiguousarray(
            np.broadcast_to(qkv_b[2 * H:][None], (128, H))).astype(bf),
        "blin2": np.asarray(lin2_b, dtype=f32).reshape(1, 1),
        "iotab": (BIG * np.arange(L, dtype=np.float64)).astype(f32)[None],
    }
    p = np.arange(128, dtype=np.float64)[:, None]
    cols = []
    for g in range(G):
        for kb in range(16):
            cols.append(BIG * (kb * 128 + p - WS[g]))
    shared["kvec"] = np.concatenate(cols, axis=1).astype(f32)
    pi = np.arange(128)[:, None]
    fi = np.arange(128)[None, :]
    shared["dtile"] = np.where(pi >= fi, 1e6, -1e6).astype(f32)
    return shared


def kernel(query, key_in, value, qkv_w, qkv_b, off_w, lin2_w, lin2_b,
           _trace=False, _tmpdir=None):
    global _BUILT
    query = np.asarray(query, dtype=np.float32)
    shared = _prep_shared(np.asarray(qkv_w, np.float32),
                          np.asarray(qkv_b, np.float32),
                          np.asarray(off_w, np.float32),
                          np.asarray(lin2_w, np.float32),
                          np.asarray(lin2_b, np.float32))
    in_maps = []
    for b in range(NCORES):
        qT = np.ascontiguousarray(query[b].T)                  # [H, L]
        m = dict(shared)
        m["qt32"] = qT
        m["qtbf"] = qT.astype(ml_dtypes.bfloat16)
        in_maps.append(m)

    if _BUILT is None:
        _BUILT = build_nc()
    kw = {}
    if _trace:
        kw = dict(trace=True, tmpdir=_tmpdir)
    res = run_bass_kernel_spmd(_BUILT, in_maps, core_ids=list(range(NCORES)), **kw)
    out = np.stack([res.results[b]["out"] for b in range(NCORES)], axis=0)
    if _trace:
        return out, res
    return out


if __name__ == "__main__":
    rng = np.random.default_rng(0)
    ins = {
        "query": rng.standard_normal((B, L, H)).astype(np.float32),
        "key_in": rng.standard_normal((B, L, H)).astype(np.float32),
        "value": rng.standard_normal((B, L, H)).astype(np.float32),
        "qkv_w": (rng.standard_normal((3 * H, H)) * 0.02).astype(np.float32),
        "qkv_b": np.zeros(3 * H, np.float32),
        "off_w": (rng.standard_normal((D1, H)) * 0.02).astype(np.float32),
        "lin2_w": (rng.standard_normal((1, D1)) * 0.02).astype(np.float32),
        "lin2_b": np.zeros(1, np.float32),
    }
    o = kernel(**ins)
    print("out", o.shape, o.dtype, np.abs(o).mean())


# revision 29
# speedup vs baseline: 1.2584x; 1.0110x over previous
"""Trainium2 Bass kernel for nn_MultiHeadAttention_28028956574019.

Sparse windowed multi-head attention, G=4 window groups, learned per-row
window offset. Data-parallel over batch: 8 NeuronCores, one batch element
per core.

Per-core device program (L=2048, H=1024, d=256 per group):
  phase 1: learned-offset path dx = L*sigmoid(lin2(relu(q) @ off_w.T))
           computed in fp32r; mask row W = BIG*(q_idx + dx) broadcast to
           [128, 2048] via gpsimd partition_broadcast.
  phase 2: Q/K projection (bf16): Q_T/K_T stored [h, l] (transposed).
  phase 3: V projection (bf16): V stored [l, h] (natural).
  phase 4: per group, per 512-wide q-strip, k-blocks descending 15..4s:
           S_T[k,q] = K_T.T@Q_T (PSUM f32), masked via
           Z = min(S, W - BIG*(k - ws)) [one DVE scalar_tensor_tensor]
           + diagonal lower-bound tile min, p = exp(SCALE2*Z) -> bf16,
           denominator via ones-matmul (PSUM accumulate), out += p.T@V.
           Normalization deferred: denom row DMA round-trips through DRAM
           to per-partition layout, out_raw * recip(denom).

All shapes hardcoded for the fixed problem size. The harness calls
kernel(**inputs) with the full (unsharded) inputs.
"""

import sys

if "/opt/trn_rl_repo" not in sys.path:
    sys.path.insert(0, "/opt/trn_rl_repo")

import numpy as np
import ml_dtypes

import concourse.bass as bass  # noqa: F401  (bass must import before bacc)
from concourse import bacc
import concourse.mybir as mybir
from concourse.tile import TileContext
from concourse.bass_utils import run_bass_kernel_spmd

dt = mybir.dt
AF = mybir.ActivationFunctionType
Alu = mybir.AluOpType

B, L, H = 8, 2048, 1024
G, D = 4, 256          # groups, per-group head dim
D1 = 256               # learned-offset hidden dim
WS = [4, 16, 64, 256]
BIG = 1.0e7
SCALE2 = 2.0 / float(np.sqrt(L))   # masked_fill+add doubles unmasked scores
NCORES = 8

_BUILT = None


def build_nc(trace_friendly=False):
    nc = bacc.Bacc("TRN2", target_bir_lowering=False, debug=False)

    # ---- I/O ----
    qt32 = nc.declare_dram_parameter("qt32", [H, L], dt.float32, isOutput=False)
    qtbf = nc.declare_dram_parameter("qtbf", [H, L], dt.bfloat16, isOutput=False)
    wqk = nc.declare_dram_parameter("wqk", [H, 2 * H], dt.bfloat16, isOutput=False)
    wv = nc.declare_dram_parameter("wv", [H, H], dt.bfloat16, isOutput=False)
    woff = nc.declare_dram_parameter("woff", [H, D1], dt.float32, isOutput=False)
    wlin2 = nc.declare_dram_parameter("wlin2", [D1, 1], dt.float32, isOutput=False)
    bqk = nc.declare_dram_parameter("bqk", [128, 16], dt.float32, isOutput=False)
    bv = nc.declare_dram_parameter("bv", [128, H], dt.bfloat16, isOutput=False)
    blin2 = nc.declare_dram_parameter("blin2", [1, 1], dt.float32, isOutput=False)
    dtile = nc.declare_dram_parameter("dtile", [128, 128], dt.float32, isOutput=False)
    iotab = nc.declare_dram_parameter("iotab", [1, L], dt.float32, isOutput=False)
    kvec = nc.declare_dram_parameter("kvec", [128, 64], dt.float32, isOutput=False)
    out = nc.declare_dram_parameter("out", [L, H], dt.float32, isOutput=True)


    with TileContext(nc) as tc:
        with tc.tile_pool(name="persist", bufs=1) as pp:
            bqk_t = pp.tile([128, 16], dt.float32, name="bqk_t")
            nc.gpsimd.dma_start(out=bqk_t[:], in_=bqk[:])
            # bf16 query (transposed), resident for phases 2-3
            qtb = []
            for i in range(8):
                t = pp.tile([128, L], dt.bfloat16, name=f"qtb{i}")
                nc.gpsimd.dma_start(out=t[:, :1024],
                                    in_=qtbf[i * 128:(i + 1) * 128, :1024])
                qtb.append(t)
            for i in range(8):
                nc.gpsimd.dma_start(out=qtb[i][:, 1024:],
                                    in_=qtbf[i * 128:(i + 1) * 128, 1024:])
            # ---- consts ----
            dt_t = pp.tile([128, 128], dt.float32, name="dt_t")
            nc.gpsimd.dma_start(out=dt_t[:], in_=dtile[:])
            kvec_t = pp.tile([128, 64], dt.float32, name="kvec_t")
            nc.gpsimd.dma_start(out=kvec_t[:], in_=kvec[:])
            blin2_t = pp.tile([1, 1], dt.float32, name="blin2_t")
            nc.gpsimd.dma_start(out=blin2_t[:], in_=blin2[:])
            bvb = pp.tile([128, H], dt.bfloat16, name="bvb")
            nc.gpsimd.dma_start(out=bvb[:], in_=bv[:])
            ones_t = pp.tile([128, 1], dt.bfloat16, name="ones_t")
            nc.vector.memset(ones_t[:], 1.0)
            one1 = pp.tile([1, 1], dt.float32, name="one1")
            nc.vector.memset(one1[:], 1.0)
            wbig = pp.tile([128, L], dt.float32, name="wbig")

            # bf16 V-projection weights, resident
            wv_t = []
            for i in range(8):
                t = pp.tile([128, H], dt.bfloat16, name=f"wv{i}")
                nc.gpsimd.dma_start(out=t[:], in_=wv[i * 128:(i + 1) * 128, :])
                wv_t.append(t)

            # persistent Q_T / K_T / V
            QT = [[pp.tile([128, L], dt.bfloat16, name=f"QT{g}{h}", tag=f"QT{g}{h}") for h in range(2)]
                  for g in range(G)]
            KT = [[pp.tile([128, L], dt.bfloat16, name=f"KT{g}{h}", tag=f"KT{g}{h}") for h in range(2)]
                  for g in range(G)]
            VT = []
            for lb in range(16):
                t = pp.tile([128, 4 * (D + 1)], dt.bfloat16, name=f"VT{lb}",
                            tag=f"VT{lb}")
                nc.vector.memset(t[:, D::D + 1], 1.0)
                VT.append(t)

            # ================= phase 2: Q/K projection =================
            with tc.tile_pool(name="p2", bufs=1) as p2, \
                 tc.tile_pool(name="ps2", bufs=3, space="PSUM") as ps2:
                for hb in range(16):
                    wt = []
                    for hin in range(8):
                        t = p2.tile([128, 128], dt.bfloat16, tag="wqk", bufs=16)
                        nc.sync.dma_start(
                            out=t[:],
                            in_=wqk[hin * 128:(hin + 1) * 128, hb * 128:(hb + 1) * 128])
                        wt.append(t)
                    g, h = (hb % 8) // 2, hb % 2
                    dest = QT[g][h] if hb < 8 else KT[g][h]
                    for s in range(4):
                        pps = ps2.tile([128, 512], dt.float32, tag="qkps")
                        for hin in range(8):
                            nc.tensor.matmul(pps[:], wt[hin][:],
                                             qtb[hin][:, s * 512:(s + 1) * 512],
                                             start=(hin == 0), stop=(hin == 7))
                        nc.scalar.activation(dest[:, s * 512:(s + 1) * 512], pps[:],
                                             AF.Identity, bias=bqk_t[:, hb:hb + 1],
                                             scale=1.0)

            # ================= phase 1: dx path =================
            with tc.tile_pool(name="p1", bufs=1) as p1, \
                 tc.tile_pool(name="ps1", bufs=2, space="PSUM") as ps1:
                woff_t = []
                for i in range(8):
                    t = p1.tile([128, D1], dt.float32r, name=f"woff{i}")
                    nc.gpsimd.dma_start(out=t[:],
                                      in_=woff[i * 128:(i + 1) * 128, :].bitcast(dt.float32r))
                    woff_t.append(t)
                wlin_t = []
                for i in range(2):
                    t = p1.tile([128, 1], dt.float32r, name=f"wlin{i}")
                    nc.gpsimd.dma_start(out=t[:],
                                      in_=wlin2[i * 128:(i + 1) * 128, :].bitcast(dt.float32r))
                    wlin_t.append(t)
                sig_row = p1.tile([1, L], dt.float32, name="sig_row")
                iotab_t = p1.tile([1, L], dt.float32, name="iotab_t")
                nc.gpsimd.dma_start(out=iotab_t[:], in_=iotab[:])

                qs_tiles = {}
                for s in range(4):
                    for hin in range(8):
                        q_s = p1.tile([128, 512], dt.float32, tag="qs", bufs=6,
                                      name="q_s")
                        nc.gpsimd.dma_start(
                            out=q_s[:],
                            in_=qt32[hin * 128:(hin + 1) * 128, s * 512:(s + 1) * 512])
                        qs_tiles[s, hin] = q_s
                def z_stage(s, t1a, t1b):
                    zps = ps1.tile([1, 512], dt.float32, tag="zps")
                    nc.tensor.matmul(zps[:], wlin_t[0][:], t1a[:], start=True, stop=False)
                    nc.tensor.matmul(zps[:], wlin_t[1][:], t1b[:], start=False, stop=True)
                    nc.scalar.activation(sig_row[:, s * 512:(s + 1) * 512], zps[:],
                                         AF.Sigmoid, bias=blin2_t[:], scale=1.0)

                zpend = None
                for s in range(4):
                    t1ps_a = ps1.tile([128, 512], dt.float32, tag="t1psa", bufs=2)
                    t1ps_b = ps1.tile([128, 512], dt.float32, tag="t1psb", bufs=2)
                    for hin in range(8):
                        q_s = qs_tiles[s, hin]
                        relu_s = p1.tile([128, 512], dt.float32r, tag="relu", bufs=2)
                        nc.vector.tensor_relu(out=relu_s[:], in_=q_s[:])
                        nc.tensor.matmul(t1ps_a[:], woff_t[hin][:, 0:128], relu_s[:],
                                         start=(hin == 0), stop=(hin == 7))
                        nc.tensor.matmul(t1ps_b[:], woff_t[hin][:, 128:256], relu_s[:],
                                         start=(hin == 0), stop=(hin == 7))
                        if hin == 3 and zpend is not None:
                            z_stage(*zpend)
                            zpend = None
                    t1a = p1.tile([128, 512], dt.float32r, tag="t1a", bufs=2)
                    t1b = p1.tile([128, 512], dt.float32r, tag="t1b", bufs=2)
                    nc.vector.tensor_copy(out=t1a[:], in_=t1ps_a[:])
                    nc.vector.tensor_copy(out=t1b[:], in_=t1ps_b[:])
                    zpend = (s, t1a, t1b)
                z_stage(*zpend)

                nc.vector.scalar_tensor_tensor(sig_row[:], sig_row[:], float(BIG * L),
                                               iotab_t[:], op0=Alu.mult, op1=Alu.add)
                nc.gpsimd.partition_broadcast(wbig[:], sig_row[:], channels=128)

            # ================= phase 3: V projection =================
            with tc.tile_pool(name="ps3", bufs=3, space="PSUM") as ps3:
                for lb in range(16):
                    for h in range(2):
                        vps = ps3.tile([128, 512], dt.float32, tag="vps")
                        for hin in range(8):
                            nc.tensor.matmul(vps[:],
                                             qtb[hin][:, lb * 128:(lb + 1) * 128],
                                             wv_t[hin][:, h * 512:(h + 1) * 512],
                                             start=(hin == 0), stop=(hin == 7))
                        for gg in range(2):
                            g2 = h * 2 + gg
                            nc.vector.tensor_tensor(
                                out=VT[lb][:, g2 * (D + 1):g2 * (D + 1) + D],
                                in0=vps[:, gg * D:(gg + 1) * D],
                                in1=bvb[:, g2 * D:(g2 + 1) * D], op=Alu.add)

            # ================= phase 4: attention =================
            with tc.tile_pool(name="p4", bufs=1) as p4, \
                 tc.tile_pool(name="pss", bufs=4, space="PSUM") as pss, \
                 tc.tile_pool(name="pso", bufs=4, space="PSUM") as pso:
                for g in range(G):
                    for s in range(4):
                        outps = [pso.tile([128, D + 1], dt.float32, tag="outps",
                                          name="outps")
                                 for _ in range(4)]
                        def consume(kb, pt, w):
                            for j in range(4):
                                if 4 * s + j <= kb:
                                    nc.tensor.matmul(
                                        outps[j][:],
                                        pt[:, j * 128:(j + 1) * 128],
                                        VT[kb][:, g * (D + 1):(g + 1) * (D + 1)],
                                        start=(kb == 15), stop=(kb == 4 * s + j))

                        pending = []
                        for kb in range(15, 4 * s - 1, -1):
                            w = 512 if kb >= 4 * s + 3 else (kb - 4 * s + 1) * 128
                            q0 = s * 512
                            sps = pss.tile([128, 512], dt.float32, tag="sps")
                            nc.tensor.matmul(sps[:, :w],
                                             KT[g][0][:, kb * 128:(kb + 1) * 128],
                                             QT[g][0][:, q0:q0 + w],
                                             start=True, stop=False)
                            nc.tensor.matmul(sps[:, :w],
                                             KT[g][1][:, kb * 128:(kb + 1) * 128],
                                             QT[g][1][:, q0:q0 + w],
                                             start=False, stop=True)
                            if len(pending) >= 3:
                                consume(*pending.pop(0))
                            z = p4.tile([128, 512], dt.bfloat16, tag="z", bufs=4)
                            nc.vector.scalar_tensor_tensor(
                                z[:, :w], wbig[:, q0:q0 + w],
                                kvec_t[:, g * 16 + kb:g * 16 + kb + 1], sps[:, :w],
                                op0=Alu.subtract, op1=Alu.min)
                            if kb <= 4 * s + 3:
                                nc.vector.tensor_tensor(out=z[:, w - 128:w],
                                                        in0=z[:, w - 128:w],
                                                        in1=dt_t[:], op=Alu.min)
                            pt = p4.tile([128, 512], dt.bfloat16, tag="pt", bufs=4)
                            nc.scalar.activation(pt[:, :w], z[:, :w], AF.Exp,
                                                 scale=SCALE2)
                            pending.append((kb, pt, w))
                        for it in pending:
                            consume(*it)
                        for j in (3, 2, 1, 0):
                            c = s * 4 + j
                            rden = p4.tile([128, 1], dt.float32, tag="rden",
                                           bufs=4, name="rden")
                            nc.vector.reciprocal(out=rden[:],
                                                 in_=outps[j][:, D:D + 1])
                            outn = p4.tile([128, D], dt.float32, tag="outn", bufs=4)
                            nc.vector.tensor_scalar(out=outn[:],
                                                    in0=outps[j][:, :D],
                                                    scalar1=rden[:], scalar2=None,
                                                    op0=Alu.mult)
                            nc.sync.dma_start(
                                out=out[c * 128:(c + 1) * 128, g * D:(g + 1) * D],
                                in_=outn[:])

    nc.finalize()
    return nc


def _prep_shared(qkv_w, qkv_b, off_w, lin2_w, lin2_b):
    f32 = np.float32
    bf = ml_dtypes.bfloat16
    qkv_wT = np.ascontiguousarray(qkv_w.T, dtype=f32)          # [H, 3H]
    shared = {
        "wqk": qkv_wT[:, :2 * H].astype(bf),
        "wv": np.ascontiguousarray(qkv_wT[:, 2 * H:]).astype(bf),
        "woff": np.ascontiguousarray(off_w.T, dtype=f32),
        "wlin2": np.ascontiguousarray(lin2_w.T, dtype=f32),
        "bqk": np.ascontiguousarray(
            qkv_b[:2 * H].reshape(16, 128).T, dtype=f32),
        "bv": np.ascontiguousarray(
            np.broadcast_to(qkv_b[2 * H:][None], (128, H))).astype(bf),
        "blin2": np.asarray(lin2_b, dtype=f32).reshape(1, 1),
        "iotab": (BIG * np.arange(L, dtype=np.float64)).astype(f32)[None],
    }
    p = np.arange(128, dtype=np.float64)[:, None]
    cols = []
    for g in range(G):
        for kb in range(16):
            cols.append(BIG * (kb * 128 + p - WS[g]))
    shared["kvec"] = np.concatenate(cols, axis=1).astype(f32)
    pi = np.arange(128)[:, None]
    fi = np.arange(128)[None, :]
    shared["dtile"] = np.where(pi >= fi, 1e6, -1e6).astype(f32)
    return shared


def kernel(query, key_in, value, qkv_w, qkv_b, off_w, lin2_w, lin2_b,
           _trace=False, _tmpdir=None):
    global _BUILT
    query = np.asarray(query, dtype=np.float32)
    shared = _prep_shared(np.asarray(qkv_w, np.float32),
                          np.asarray(qkv_b, np.float32),
                          np.asarray(off_w, np.float32),
                          np.asarray(lin2_w, np.float32),
                          np.asarray(lin2_b, np.float32))
    in_maps = []
    for b in range(NCORES):
        qT = np.ascontiguousarray(query[b].T)                  # [H, L]
        m = dict(shared)
        m["qt32"] = qT
        m["qtbf"] = qT.astype(ml_dtypes.bfloat16)
        in_maps.append(m)

    if _BUILT is None:
        _BUILT = build_nc()
    kw = {}
    if _trace:
        kw = dict(trace=True, tmpdir=_tmpdir)
    res = run_bass_kernel_spmd(_BUILT, in_maps, core_ids=list(range(NCORES)), **kw)
    out = np.stack([res.results[b]["out"] for b in range(NCORES)], axis=0)
    if _trace:
        return out, res
    return out


if __name__ == "__main__":
    rng = np.random.default_rng(0)
    ins = {
        "query": rng.standard_normal((B, L, H)).astype(np.float32),
        "key_in": rng.standard_normal((B, L, H)).astype(np.float32),
        "value": rng.standard_normal((B, L, H)).astype(np.float32),
        "qkv_w": (rng.standard_normal((3 * H, H)) * 0.02).astype(np.float32),
        "qkv_b": np.zeros(3 * H, np.float32),
        "off_w": (rng.standard_normal((D1, H)) * 0.02).astype(np.float32),
        "lin2_w": (rng.standard_normal((1, D1)) * 0.02).astype(np.float32),
        "lin2_b": np.zeros(1, np.float32),
    }
    o = kernel(**ins)
    print("out", o.shape, o.dtype, np.abs(o).mean())
